# revision 13
# baseline (speedup 1.0000x reference)
"""AttnBlock (GroupNorm + single-head self-attention + residual) on 8 TRN2 cores.

Sharding: data-parallel over batch B=8 -> one [64,64,128] image per core.
Per-core kernel layout notes:
  - xT/hT/qT/kT are [C=128 partitions, N=4096 free] (channels on partitions).
  - Scores are computed directly transposed: sT[k, q] = k_chunk @ qT so the
    probability matrix lands in [k-partition, q-free] layout, which is what
    the PV matmul needs (contraction over k on partitions).
  - Softmax skips max-subtraction (scores are O(1) here; exp can't overflow
    fp32) and the denominator Z is accumulated with an all-ones stationary
    matmul alongside PV. Z is then moved to q-partition layout with 8 tiny
    N=1 matmuls (lhsT = Z row-block, rhs = e0) so the normalization becomes
    a cheap per-partition scalar multiply in the epilogue.
  - The (q-block, k-chunk) loop is software-pipelined: score matmuls + exp
    run two steps ahead of the PV/Z accumulation so the PE never waits on
    the activation engine.
  - Matmuls run as float32r (fp32 bits, ~1 cycle/row on the PE at N=512).
"""

import sys

for _p in ("/opt/trn_rl_repo",):
    if _p not in sys.path:
        sys.path.insert(0, _p)

import numpy as np

import concourse.bass as bass
import concourse.tile as tile
from concourse import bacc, mybir
from concourse.bass_utils import run_bass_kernel_spmd

B, H, W, C = 8, 64, 64, 128
N = H * W  # 4096 positions per image
GROUPS = 32
GSIZE = C // GROUPS  # 4
EPS = 1e-6
NCORES = 8
P = 128
NT = N // P  # 32 position tiles / k-chunks
QW = 1024  # q-block width of the attention main loop
NQQ = N // QW  # 4
NSUB = QW // P  # 8 q-subtiles per block
SCALE = C ** -0.5

F32 = mybir.dt.float32
F32R = mybir.dt.float32r
BF16 = mybir.dt.bfloat16

# "f32r" | "f32" | "bf16" : dtype used by the big matmuls
MM_MODE = "f32r"


def _mm_store_dt():
    # dtype of tiles feeding the big matmuls; producers round on write
    # (the BIR verifier requires f32r matmul operands to be produced as f32r)
    return {"bf16": BF16, "f32r": F32R, "f32": F32}[MM_MODE]


def build_nc():
    nc = bacc.Bacc("TRN2", target_bir_lowering=False, debug=False)
    mdt = _mm_store_dt()

    x_d = nc.dram_tensor("x", [N, C], F32, kind="ExternalInput")
    wq_d = nc.dram_tensor("wq", [C, C], F32, kind="ExternalInput")
    wk_d = nc.dram_tensor("wk", [C, C], F32, kind="ExternalInput")
    wv_d = nc.dram_tensor("wv", [C, C], F32, kind="ExternalInput")
    wo_d = nc.dram_tensor("wo", [C, C], F32, kind="ExternalInput")
    bq_d = nc.dram_tensor("bq", [C], F32, kind="ExternalInput")
    bk_d = nc.dram_tensor("bk", [C], F32, kind="ExternalInput")
    bv_d = nc.dram_tensor("bv", [C], F32, kind="ExternalInput")
    bo_d = nc.dram_tensor("bo", [C], F32, kind="ExternalInput")
    gns_d = nc.dram_tensor("gn_scale", [C], F32, kind="ExternalInput")
    gnb_d = nc.dram_tensor("gn_bias", [C], F32, kind="ExternalInput")
    ident_d = nc.dram_tensor("ident", [P, P], F32, kind="ExternalInput")
    gmask_d = nc.dram_tensor("gmask", [P, P], F32, kind="ExternalInput")
    ones_d = nc.dram_tensor("onesm", [P, P], F32, kind="ExternalInput")
    out_d = nc.dram_tensor("out", [N, C], F32, kind="ExternalOutput")

    # DRAM views with positions split into [tile, partition]
    x_tiled = x_d.rearrange("(t p) c -> p t c", p=P)
    out_tiled = out_d.rearrange("(t p) c -> p t c", p=P)

    def col(ap_1d):
        # [C] dram -> [C, 1] partition-column view
        return ap_1d.unsqueeze(1)

    def brow(ap_1d):
        # [C] dram -> [128, C] partition-broadcast view (step-0 partition dim)
        return bass.AP(
            tensor=ap_1d.tensor, offset=ap_1d.offset, ap=[[0, P]] + list(ap_1d.ap)
        )

    with tile.TileContext(nc) as tc:
        with (
            tc.tile_pool(name="persist", bufs=1) as data,
            tc.tile_pool(name="small", bufs=1) as small,
        ):
            # ---- persistent SBUF tiles ----
            x_all = data.tile([P, NT, C], F32)  # x in [pos-in-tile, tile, C]
            xT = data.tile([P, N], F32)  # x transposed: [C, pos]
            hT = data.tile([P, N], mdt)  # groupnorm output, [C, pos]
            qT = data.tile([P, N], mdt)
            kT = data.tile([P, N], mdt)
            v_all = data.tile([P, NT, C], mdt)  # v in [pos-in-tile, tile, C]

            wq_s = small.tile([C, C], mdt)
            wk_s = small.tile([C, C], mdt)
            wv_s = small.tile([C, C], mdt)
            wo_s = small.tile([C, C], mdt)
            ident_s = small.tile([P, P], F32)
            gmask_s = small.tile([P, P], F32)
            ones_s = small.tile([P, P], mdt)
            bq_s = small.tile([C, 1], F32)
            bk_s = small.tile([C, 1], F32)
            bv_r = small.tile([P, C], F32)  # bv broadcast to all partitions
            bo_r = small.tile([P, C], F32)
            gns_s = small.tile([C, 1], F32)
            gnb_s = small.tile([C, 1], F32)
            eps_s = small.tile([C, 1], F32)

            # sync (HWDGE) queue: ident first (gates transposes), then x
            # chunks; everything else rides the gpsimd (SWDGE) queue so the
            # two DMA paths run concurrently at startup.
            nc.sync.dma_start(ident_s[:], ident_d[:])
            XCH = 4
            for ci in range(XCH):
                ts = slice(ci * NT // XCH, (ci + 1) * NT // XCH)
                nc.sync.dma_start(x_all[:, ts, :], x_tiled[:, ts, :])

            def ld2(dst, src):
                # SWDGE load; for f32r stage as fp32 then round on DVE (the
                # BIR verifier requires f32r matmul operands produced as f32r)
                if MM_MODE == "f32r":
                    stg = small.tile(list(dst.shape), F32, tag="wstage")
                    nc.gpsimd.dma_start(stg[:], src)
                    nc.vector.tensor_copy(dst[:], stg[:])
                else:
                    nc.gpsimd.dma_start(dst[:], src)

            ld2(wq_s[:], wq_d[:])
            ld2(wk_s[:], wk_d[:])
            ld2(wv_s[:], wv_d[:])
            ld2(wo_s[:], wo_d[:])
            ld2(ones_s[:], ones_d[:])
            nc.gpsimd.dma_start(gmask_s[:], gmask_d[:])
            nc.gpsimd.dma_start(bq_s[:], col(bq_d[:]))
            nc.gpsimd.dma_start(bk_s[:], col(bk_d[:]))
            nc.gpsimd.dma_start(bv_r[:], brow(bv_d[:]))
            nc.gpsimd.dma_start(bo_r[:], brow(bo_d[:]))
            nc.gpsimd.dma_start(gns_s[:], col(gns_d[:]))
            nc.gpsimd.dma_start(gnb_s[:], col(gnb_d[:]))
            nc.vector.memset(eps_s[:], EPS)

            # ---- phase 1: transpose x into [C, pos]; bn_stats interleaved ----
            stats = small.tile([P, 8, nc.vector.BN_STATS_DIM], F32)
            with tc.tile_pool(name="tp", bufs=3, space="PSUM") as tpsum:
                for i in range(NT):
                    pt = tpsum.tile([P, P], F32, tag="tp")
                    nc.tensor.transpose(pt[:], x_all[:, i, :], ident_s[:])
                    nc.any.tensor_copy(xT[:, i * P : (i + 1) * P], pt[:])
                    if i % 4 == 3:
                        j = i // 4
                        nc.vector.bn_stats(
                            out=stats[:, j, :], in_=xT[:, j * 512 : (j + 1) * 512]
                        )

                # x_all += bo (residual add uses x_all later)
                nc.vector.tensor_add(
                    x_all[:],
                    x_all[:],
                    bo_r[:, None, :].to_broadcast((P, NT, C)),
                )

                # ---- phase 2: group norm ----
                mv = small.tile([P, nc.vector.BN_AGGR_DIM], F32)
                nc.vector.bn_aggr(out=mv[:], in_=stats[:])
                # per-channel [mean, E[x^2]] -> group-averaged via mask matmul
                st2 = small.tile([P, 2], F32)
                nc.vector.tensor_copy(st2[:, 0:1], mv[:, 0:1])
                msq = small.tile([P, 1], F32)
                nc.vector.tensor_mul(msq[:], mv[:, 0:1], mv[:, 0:1])
                nc.vector.tensor_add(st2[:, 1:2], mv[:, 1:2], msq[:])
                gpsum = tpsum.tile([P, 2], F32, tag="tp")
                nc.tensor.matmul(gpsum[:], gmask_s[:], st2[:])
                gstat = small.tile([P, 2], F32)
                nc.vector.tensor_copy(gstat[:], gpsum[:])

                # var_g = E_g[x^2] - mean_g^2 ; rstd = 1/sqrt(var_g + eps)
                varg = small.tile([P, 1], F32)
                nc.vector.tensor_mul(varg[:], gstat[:, 0:1], gstat[:, 0:1])
                nc.vector.tensor_tensor(
                    varg[:], gstat[:, 1:2], varg[:], mybir.AluOpType.subtract
                )
                nc.scalar.activation(
                    out=varg[:],
                    in_=varg[:],
                    func=mybir.ActivationFunctionType.Sqrt,
                    bias=eps_s[:],
                    scale=1.0,
                )
                rstd = small.tile([P, 1], F32)
                nc.vector.reciprocal(rstd[:], varg[:])
                # h = x * A + Bc with A = rstd*scale, Bc = bias - mean*A
                A_s = small.tile([P, 1], F32)
                nc.vector.tensor_mul(A_s[:], rstd[:], gns_s[:])
                mA = small.tile([P, 1], F32)
                nc.vector.tensor_mul(mA[:], gstat[:, 0:1], A_s[:])
                Bc_s = small.tile([P, 1], F32)
                nc.vector.tensor_tensor(
                    Bc_s[:], gnb_s[:], mA[:], mybir.AluOpType.subtract
                )
                # hT in 8 chunks so projections can start early
                for j in range(8):
                    sl = slice(j * 512, (j + 1) * 512)
                    nc.vector.tensor_scalar(
                        out=hT[:, sl],
                        in0=xT[:, sl],
                        scalar1=A_s[:],
                        scalar2=Bc_s[:],
                        op0=mybir.AluOpType.mult,
                        op1=mybir.AluOpType.add,
                    )

            # ---- phase 3: projections qT/kT [C,N], v [pos,C] ----
            with (
                tc.tile_pool(name="pq", bufs=3, space="PSUM") as pqpool,
                tc.tile_pool(name="pv", bufs=3, space="PSUM") as pvpool,
            ):
                # emission order favors what the attention loop needs first:
                # qT block 0 (j=0,1), all of kT, all of v, then the rest of qT
                def emit_q(j):
                    sl = slice(j * 512, (j + 1) * 512)
                    pq = pqpool.tile([P, 512], F32, tag="pq")
                    nc.tensor.matmul(pq[:], wq_s[:], hT[:, sl])
                    nc.vector.tensor_scalar_add(qT[:, sl], pq[:], bq_s[:])

                for j in range(2):
                    emit_q(j)
                for j in range(8):
                    sl = slice(j * 512, (j + 1) * 512)
                    pk = pqpool.tile([P, 512], F32, tag="pq")
                    nc.tensor.matmul(pk[:], wk_s[:], hT[:, sl])
                    nc.vector.tensor_scalar_add(kT[:, sl], pk[:], bk_s[:])
                for i in range(NT):
                    pv = pvpool.tile([P, C], F32, tag="pv")
                    nc.tensor.matmul(pv[:], hT[:, i * P : (i + 1) * P], wv_s[:])
                    nc.any.tensor_add(v_all[:, i, :], pv[:], bv_r[:])
                for j in range(2, 8):
                    emit_q(j)

            # ---- phase 4: attention, software-pipelined over 128 steps ----
            with (
                tc.tile_pool(name="sT", bufs=2, space="PSUM") as sTpool,
                tc.tile_pool(name="oT", bufs=1, space="PSUM") as oTpool,
                tc.tile_pool(name="Zp", bufs=1, space="PSUM") as zpool,
                tc.tile_pool(name="pexp", bufs=4) as pexppool,
                tc.tile_pool(name="onorm", bufs=2) as onormpool,
                tc.tile_pool(name="ostage", bufs=2) as ostagepool,
            ):
                NSTEP = NQQ * NT  # 128
                pexp_tiles = {}
                psum_oT = {}
                psum_Z = {}
                tail_state = {}

                def emit_scores(step):
                    qq, kc = divmod(step, NT)
                    q0 = qq * QW
                    ksl = slice(kc * P, (kc + 1) * P)
                    psum_sT = sTpool.tile([P, QW], F32, tag="sT")
                    for h2 in range(QW // 512):
                        qsl = slice(q0 + h2 * 512, q0 + (h2 + 1) * 512)
                        nc.tensor.matmul(
                            psum_sT[:, h2 * 512 : (h2 + 1) * 512],
                            kT[:, ksl],
                            qT[:, qsl],
                        )
                    pexp = pexppool.tile([P, QW], _mm_store_dt(), tag="pexp")
                    nc.scalar.activation(
                        out=pexp[:],
                        in_=psum_sT[:],
                        func=mybir.ActivationFunctionType.Exp,
                        scale=SCALE,
                    )
                    pexp_tiles[step] = pexp

                def emit_pvz(step):
                    qq, kc = divmod(step, NT)
                    if kc == 0:
                        psum_oT[qq] = oTpool.tile(
                            [P, QW], F32, tag="oT", name=f"psum_oT_{qq}"
                        )
                        psum_Z[qq] = zpool.tile(
                            [P, QW], F32, tag="Z", name=f"psum_Z_{qq}"
                        )
                    pexp = pexp_tiles.pop(step)
                    first, last = kc == 0, kc == NT - 1
                    for h2 in range(QW // 512):
                        hsl = slice(h2 * 512, (h2 + 1) * 512)
                        nc.tensor.matmul(
                            psum_oT[qq][:, hsl],
                            v_all[:, kc, :],
                            pexp[:, hsl],
                            start=first,
                            stop=last,
                        )
                        nc.tensor.matmul(
                            psum_Z[qq][:, hsl],
                            ones_s[:],
                            pexp[:, hsl],
                            start=first,
                            stop=last,
                        )

                def emit_tail_head(qq):
                    """Z/oT PSUM -> SBUF, Z to q-partition layout, 1/Z."""
                    poT, pZ = psum_oT.pop(qq), psum_Z.pop(qq)
                    # Z (all rows identical) -> SBUF, then extract to
                    # q-partition layout via 8 tiny N=1 matmuls into a view
                    # of the (now free) Z psum tile.
                    Z_sb = onormpool.tile([P, QW], F32, tag="zsb", name=f"Zsb{qq}")
                    nc.vector.tensor_copy(Z_sb[:], pZ[:])
                    zq_psum = pZ[:, 0:NSUB]
                    for s in range(NSUB):
                        nc.tensor.matmul(
                            zq_psum[:, s : s + 1],
                            Z_sb[:, s * P : (s + 1) * P],
                            ident_s[:, 0:1],
                        )
                    rzq = onormpool.tile([P, NSUB], F32, tag="rzq", name=f"rzq{qq}")
                    nc.vector.reciprocal(rzq[:], zq_psum)
                    oT_sb = onormpool.tile(
                        [P, QW], _mm_store_dt(), tag="on", name=f"oTsb{qq}"
                    )
                    nc.vector.tensor_copy(oT_sb[:], poT[:])
                    ostage = ostagepool.tile(
                        [P, NSUB, C], F32, tag="os", name=f"ost{qq}"
                    )
                    tail_state[qq] = (poT, oT_sb, rzq, ostage)

                def emit_tail_sub(qq, s, last):
                    """One q-subtile: out-projection (into freed oT psum
                    banks) + normalized residual add; DMA after the last."""
                    poT, oT_sb, rzq, ostage = tail_state[qq]
                    pop = poT[:, s * P : (s + 1) * P]
                    nc.tensor.matmul(pop, oT_sb[:, s * P : (s + 1) * P], wo_s[:])
                    # out = attn/Z + (x + bo)
                    nc.vector.scalar_tensor_tensor(
                        out=ostage[:, s, :],
                        in0=pop,
                        scalar=rzq[:, s : s + 1],
                        in1=x_all[:, qq * NSUB + s, :],
                        op0=mybir.AluOpType.mult,
                        op1=mybir.AluOpType.add,
                    )
                    if last:
                        del tail_state[qq]
                        nc.sync.dma_start(
                            out_tiled[:, qq * NSUB : (qq + 1) * NSUB, :], ostage[:]
                        )

                LOOKAHEAD = 2
                for step in range(LOOKAHEAD):
                    emit_scores(step)
                for step in range(NSTEP):
                    qq, kc = divmod(step, NT)
                    emit_pvz(step)
                    if step + LOOKAHEAD < NSTEP:
                        emit_scores(step + LOOKAHEAD)
                    if kc == NT - 1:
                        emit_tail_head(qq)
                    # spread the previous block's out-projection across steps
                    if qq >= 1 and kc < NSUB:
                        emit_tail_sub(qq - 1, kc, last=kc == NSUB - 1)
                # last block's tail: alternate PSUM banks so the DVE reads
                # don't serialize against the PE writes (bank-level deps)
                for s in (0, 4, 1, 5, 2, 6, 3, 7):
                    emit_tail_sub(NQQ - 1, s, last=s == 7)

    nc.compile()
    return nc


_NC_CACHE = {}


def _get_nc():
    key = MM_MODE
    if key not in _NC_CACHE:
        _NC_CACHE[key] = build_nc()
    return _NC_CACHE[key]


def make_in_maps(**inputs):
    x = np.ascontiguousarray(np.asarray(inputs["x"], dtype=np.float32))
    ident = np.eye(P, dtype=np.float32)
    gmask = (
        np.kron(np.eye(GROUPS, dtype=np.float32), np.ones((GSIZE, GSIZE), np.float32))
        / GSIZE
    )
    onesm = np.ones((P, P), dtype=np.float32)
    shared = {
        "wq": np.asarray(inputs["wq"], np.float32),
        "wk": np.asarray(inputs["wk"], np.float32),
        "wv": np.asarray(inputs["wv"], np.float32),
        "wo": np.asarray(inputs["wo"], np.float32),
        "bq": np.asarray(inputs["bq"], np.float32),
        "bk": np.asarray(inputs["bk"], np.float32),
        "bv": np.asarray(inputs["bv"], np.float32),
        "bo": np.asarray(inputs["bo"], np.float32),
        "gn_scale": np.asarray(inputs["gn_scale"], np.float32),
        "gn_bias": np.asarray(inputs["gn_bias"], np.float32),
        "ident": ident,
        "gmask": gmask,
        "onesm": onesm,
    }
    return [{"x": x[b].reshape(N, C), **shared} for b in range(B)]


def kernel(**inputs):
    nc = _get_nc()
    in_maps = make_in_maps(**inputs)
    res = run_bass_kernel_spmd(nc, in_maps, core_ids=list(range(NCORES)))
    out = np.stack([res.results[b]["out"] for b in range(B)], axis=0)
    return out.reshape(B, H, W, C).astype(np.float32)


if __name__ == "__main__":
    rng = np.random.default_rng(0)
    ins = {
        "x": rng.standard_normal((B, H, W, C), dtype=np.float32),
        "gn_scale": np.ones(C, np.float32),
        "gn_bias": np.zeros(C, np.float32),
    }
    for w in ("wq", "wk", "wv", "wo"):
        ins[w] = rng.standard_normal((C, C), dtype=np.float32) * SCALE
    for b in ("bq", "bk", "bv", "bo"):
        ins[b] = np.zeros(C, np.float32)
    o = kernel(**ins)
    print("out", o.shape, o.dtype, float(np.abs(o).max()))
